# revision 1
# baseline (speedup 1.0000x reference)
"""Local causal (sliding-window) attention kernel for Trainium2, SPMD over 8 cores.

Problem: states [4, 4096, 1024] f32; q/k/v = states @ W*.T + b*; each query t
attends keys t-8..t (window=8), softmax over valid positions, out = attn @ v.

Sharding: data-parallel, 8 shards = 4 batches x 2 sequence halves (2048 queries
each). The host supplies each shard's states pre-transposed to [H, 2056] with an
8-row halo (zero-padded at sequence start; masked out via the additive mask).

Score reformulation (saves one full GEMM): q.k = x_t^T A x_k + g.x_t + w.x_k
+ c0 with A = (Wq/sqrt(H))^T Wk precomputed on host. The device computes
Y = A @ X (one GEMM) instead of both Q and K projections; X itself serves as
the score lhsT. The rank-1 terms u[k] = w.x_k and gx[t] = g.x_t are host
GEMVs accumulated into the score PSUM via two tiny K=1 matmuls; c0 is folded
into the additive masks.

Per-core device plan (bf16 matmuls, f32 PSUM):
  - Y[H,2056] (halo cols from host), v[2048+tail,H] row-major (tail rows from
    host). All ragged matmul chunks eliminated.
  - Attention per 128-query tile j: S[128,136] = X_q^T Y_span (8 accum matmuls
    + 2 rank-1); masked softmax along free dim with fused exp+rowsum; P
    transposed via PE identity matmul; out = P @ V_span (136-contraction split
    128+8); 1/rowsum applied on the PSUM->SBUF copy.
"""

import numpy as np
import ml_dtypes

import concourse.bacc as bacc
import concourse.mybir as mybir
import concourse.tile as tile
from concourse.bass_utils import run_bass_kernel_spmd

B, T, H = 4, 4096, 1024
NCORES = 8
TC = T // 2            # queries per core
HALO = 8               # window size
TH = TC + HALO         # shard cols incl. halo
SPAN = 128 + HALO      # key span per 128-query tile
NT = TC // 128         # query tiles per core
HC = H // 128          # 128-row chunks of H
F32 = mybir.dt.float32
BF16 = mybir.dt.bfloat16
BF = ml_dtypes.bfloat16
AF = mybir.ActivationFunctionType

_cache = {}


def _emit(nc, tc, aps, pools):
    (x_d, a_d, wv_d, bv_d, m0_d, mr_d, id_d, u_d, on_d, yh_d,
     vt_d, out_d) = aps
    consts, xw, acts, psP, psS, psT, psO, attn = pools

    bv_t = consts.tile([128, H], F32, tag="bv", name="bv_t")
    m0_t = consts.tile([128, SPAN], F32, tag="m0", name="m0_t")
    mr_t = consts.tile([128, SPAN], F32, tag="mr", name="mr_t")
    id_t = consts.tile([128, 128], BF16, tag="id", name="id_t")
    u_t = consts.tile([1, TH], BF16, tag="u", name="u_t")
    on_t = consts.tile([1, 128], BF16, tag="on", name="on_t")

    xt = [xw.tile([128, TH], BF16, tag=f"x{c}", name=f"x{c}") for c in range(HC)]
    at = [xw.tile([128, H], BF16, tag=f"a{c}", name=f"a{c}") for c in range(HC)]
    wvt = [xw.tile([128, H], BF16, tag=f"wv{c}", name=f"wv{c}") for c in range(HC)]
    yt = [acts.tile([128, TH], BF16, tag=f"y{c}", name=f"y{c}") for c in range(HC)]
    vt = [acts.tile([128, H], BF16, tag=f"v{j}", name=f"v{j}")
          for j in range(NT)]
    vtail = acts.tile([HALO, H], BF16, tag="vtail", name="vtail")

    # DMA issue order = data-need order. First Y group needs x seg0 + a;
    # first V group additionally wv; attention group 0 needs consts.
    for c in range(HC):   # x cols 0..519 (halo + first 512-col segment)
        nc.gpsimd.dma_start(xt[c][:, 0:HALO + 512],
                            x_d[c * 128:(c + 1) * 128, 0:HALO + 512])
    for c in range(HC):
        nc.gpsimd.dma_start(at[c][:], a_d[c * 128:(c + 1) * 128, :])
    for c in range(HC):
        nc.gpsimd.dma_start(wvt[c][:], wv_d[c * 128:(c + 1) * 128, :])
    nc.gpsimd.dma_start(bv_t[:], bv_d[:])
    nc.gpsimd.dma_start(m0_t[:], m0_d[:])
    nc.gpsimd.dma_start(mr_t[:], mr_d[:])
    nc.gpsimd.dma_start(id_t[:], id_d[:])
    nc.gpsimd.dma_start(u_t[:], u_d[:])
    nc.gpsimd.dma_start(on_t[:], on_d[:])
    for c in range(HC):   # Y halo cols from host
        nc.gpsimd.dma_start(yt[c][:, 0:HALO], yh_d[c * 128:(c + 1) * 128, :])
    for seg in range(1, TC // 512):   # remaining x column segments
        lo = HALO + seg * 512
        for c in range(HC):
            nc.gpsimd.dma_start(xt[c][:, lo:lo + 512],
                                x_d[c * 128:(c + 1) * 128, lo:lo + 512])
    nc.gpsimd.dma_start(vtail[:], vt_d[:])

    def emit_y(t4):
        off = HALO + t4 * 512
        for hc in range(HC):
            ps = psP.tile([128, 512], F32, tag="ps", name="psy")
            for c in range(HC):
                nc.tensor.matmul(
                    ps[:], at[c][:, hc * 128:(hc + 1) * 128],
                    xt[c][:, off: off + 512],
                    start=(c == 0), stop=(c == HC - 1))
            nc.scalar.copy(yt[hc][:, off: off + 512], ps[:])

    def emit_v(j):
        for hh in range(2):
            ps = psP.tile([128, 512], F32, tag="ps", name="psv")
            for c in range(HC):
                nc.tensor.matmul(
                    ps[:], xt[c][:, j * 128: (j + 1) * 128],
                    wvt[c][:, hh * 512:(hh + 1) * 512],
                    start=(c == 0), stop=(c == HC - 1))
            nc.vector.tensor_add(
                vt[j][:, hh * 512:(hh + 1) * 512], ps[:],
                bv_t[:, hh * 512:(hh + 1) * 512])

    def emit_attn(j):
        s_ps = psS.tile([128, SPAN], F32, tag="s", name="s_ps")
        for c in range(HC):
            nc.tensor.matmul(
                s_ps[:], xt[c][:, HALO + j * 128: HALO + (j + 1) * 128],
                yt[c][:, j * 128: j * 128 + SPAN],
                start=(c == 0), stop=False)
        nc.tensor.matmul(s_ps[:], on_t[:, 0:128],
                         u_t[:, j * 128: j * 128 + SPAN],
                         start=False, stop=True)
        s_sb = attn.tile([128, SPAN], F32, tag="ssb", name="s_sb")
        nc.vector.tensor_add(s_sb[:], s_ps[:],
                             (m0_t if j == 0 else mr_t)[:])
        negmax = attn.tile([128, 1], F32, tag="nm", name="negmax")
        nc.vector.reduce_max(negmax[:], s_sb[:],
                             axis=mybir.AxisListType.X, negate=True)
        p_bf = attn.tile([128, SPAN], BF16, tag="p", name="p_bf")
        rowsum = attn.tile([128, 1], F32, tag="rs", name="rowsum")
        nc.scalar.activation(p_bf[:], s_sb[:], AF.Exp,
                             bias=negmax[:], scale=1.0,
                             accum_out=rowsum[:])
        rinv = attn.tile([128, 1], F32, tag="ri", name="rinv")
        nc.vector.reciprocal(rinv[:], rowsum[:])

        pt_ps = psT.tile([128, 256], BF16, tag="pt", name="pt_ps")
        nc.tensor.transpose(pt_ps[:, 0:128], p_bf[:, 0:128], id_t[:])
        nc.tensor.transpose(pt_ps[:HALO, 128:256], p_bf[:, 128:SPAN], id_t[:])
        pta_sb = attn.tile([128, 128], BF16, tag="ptas", name="pta_sb")
        ptb_sb = attn.tile([HALO, 128], BF16, tag="ptbs", name="ptb_sb")
        nc.scalar.copy(pta_sb[:], pt_ps[:, 0:128])
        nc.vector.tensor_copy(ptb_sb[:], pt_ps[:HALO, 128:256])

        vnext = vtail if j == NT - 1 else vt[j + 1]
        out_sb = attn.tile([128, H], F32, tag="osb", name="out_sb")
        for hh in range(2):
            o_ps = psO.tile([128, 512], F32, tag="o", name="o_ps")
            nc.tensor.matmul(o_ps[:], pta_sb[:],
                             vt[j][:, hh * 512:(hh + 1) * 512],
                             start=True, stop=False)
            nc.tensor.matmul(o_ps[:], ptb_sb[:],
                             vnext[:HALO, hh * 512:(hh + 1) * 512],
                             start=False, stop=True)
            nc.scalar.activation(
                out_sb[:, hh * 512:(hh + 1) * 512], o_ps[:],
                AF.Copy, bias=0.0, scale=rinv[:])
        nc.sync.dma_start(out_d[j * 128:(j + 1) * 128, :], out_sb[:])

    # Interleave: after Y t-chunk t4 and v tiles 4*t4..4*t4+3, attention
    # tiles j <= 4*t4+2 have everything they need (span fits in Y cols
    # < 8+512*(t4+1); PV needs v[j+1]).
    for t4 in range(TC // 512):
        emit_y(t4)
        for j in range(4 * t4, 4 * t4 + 4):
            emit_v(j)
        for j in range(max(0, 4 * t4 - 1), 4 * t4 + 3):
            emit_attn(j)
    emit_attn(NT - 1)


def _build(loop_reps=None, trace_sim=False):
    key = ("nc", loop_reps, trace_sim)
    if key in _cache:
        return _cache[key]
    nc = bacc.Bacc("TRN2", target_bir_lowering=False, debug=False,
                   num_devices=NCORES)

    aps = (
        nc.dram_tensor("x", [H, TH], BF16, kind="ExternalInput").ap(),
        nc.dram_tensor("a", [H, H], BF16, kind="ExternalInput").ap(),
        nc.dram_tensor("wv", [H, H], BF16, kind="ExternalInput").ap(),
        nc.dram_tensor("bv", [128, H], F32, kind="ExternalInput").ap(),
        nc.dram_tensor("m0", [128, SPAN], F32, kind="ExternalInput").ap(),
        nc.dram_tensor("mr", [128, SPAN], F32, kind="ExternalInput").ap(),
        nc.dram_tensor("ident", [128, 128], BF16, kind="ExternalInput").ap(),
        nc.dram_tensor("u", [1, TH], BF16, kind="ExternalInput").ap(),
        nc.dram_tensor("ones", [1, 128], BF16, kind="ExternalInput").ap(),
        nc.dram_tensor("yhalo", [H, HALO], BF16, kind="ExternalInput").ap(),
        nc.dram_tensor("vtail", [HALO, H], BF16, kind="ExternalInput").ap(),
        nc.dram_tensor("out", [TC, H], F32, kind="ExternalOutput").ap(),
    )

    with tile.TileContext(nc, trace_sim=trace_sim) as tc:
        with (
            tc.tile_pool(name="consts", bufs=1) as consts,
            tc.tile_pool(name="xw", bufs=1) as xw,
            tc.tile_pool(name="acts", bufs=1) as acts,
            tc.tile_pool(name="psP", bufs=2, space="PSUM") as psP,
            tc.tile_pool(name="psS", bufs=2, space="PSUM") as psS,
            tc.tile_pool(name="psT", bufs=1, space="PSUM") as psT,
            tc.tile_pool(name="psO", bufs=3, space="PSUM") as psO,
            tc.tile_pool(name="attn", bufs=3) as attn,
        ):
            pools = (consts, xw, acts, psP, psS, psT, psO, attn)
            if loop_reps:
                with tc.For_i(0, loop_reps, 1):
                    _emit(nc, tc, aps, pools)
            else:
                _emit(nc, tc, aps, pools)

    nc.compile()
    _cache[key] = nc
    return nc


def _host_inputs(states, Wq, bq, Wk, bk, Wv, bv):
    """Shared (per-run) host-side tensor prep."""
    scale = 1.0 / np.sqrt(H)
    Wq = np.asarray(Wq, np.float32)
    Wk = np.asarray(Wk, np.float32)
    Wv = np.asarray(Wv, np.float32)
    bq = np.asarray(bq, np.float32)
    bk = np.asarray(bk, np.float32)
    bv = np.asarray(bv, np.float32)
    Wqs = Wq * scale
    # A = Wqs.T @ Wk ; device lhsT layout needs A.T = Wk.T @ Wqs
    at_h = np.ascontiguousarray(Wk.T @ Wqs).astype(BF)
    # per-key rank-1 vector; per-query term and constants cancel in softmax
    wt_h = Wk.T @ (bq * scale)
    wv_h = np.ascontiguousarray(Wv.T).astype(BF)
    bv_h = np.ascontiguousarray(np.broadcast_to(bv, (128, H)))
    m = np.arange(128)[:, None]
    n = np.arange(SPAN)[None, :]
    band = (n >= m) & (n <= m + HALO)
    mr_h = np.where(band, 0.0, -30000.0).astype(np.float32)
    m0_h = np.where(band & (n >= HALO), 0.0, -30000.0).astype(np.float32)
    id_h = np.eye(128).astype(BF)
    on_h = np.ones((1, 128), dtype=BF)
    return at_h, wt_h, wv_h, bv_h, m0_h, mr_h, id_h, on_h, bv


def _shard_maps(states, hosts):
    at_h, wt_h, wv_h, bv_h, m0_h, mr_h, id_h, on_h, bv = hosts
    a_f = at_h.astype(np.float32)      # [hin, hout] = A.T in bf16 precision
    wv_f = wv_h.astype(np.float32)
    in_maps = []
    for i in range(NCORES):
        b, hf = i // 2, i % 2
        xs = np.zeros((TH, H), np.float32)
        if hf == 0:
            xs[HALO:] = states[b, 0:TC]
        else:
            xs[:] = states[b, TC - HALO: 2 * TC]
        x_h = np.ascontiguousarray(xs.T).astype(BF)   # [H, TH]
        x_f = x_h.astype(np.float32)
        u_h = (wt_h @ x_f).astype(BF).reshape(1, TH)
        yh_h = (a_f.T @ x_f[:, :HALO]).astype(BF)      # [H, 8]
        vtail_h = (x_f[:, TC:].T @ wv_f + bv).astype(BF)  # [8, H]
        in_maps.append({
            "x": x_h, "a": at_h, "wv": wv_h, "bv": bv_h,
            "m0": (m0_h if hf == 0 else mr_h), "mr": mr_h, "ident": id_h,
            "u": u_h, "ones": on_h, "yhalo": yh_h,
            "vtail": vtail_h,
        })
    return in_maps


def kernel(states, Wq, bq, Wk, bk, Wv, bv, window):
    assert int(window) == HALO
    states = np.asarray(states, np.float32)
    nc = _build()
    hosts = _host_inputs(states, Wq, bq, Wk, bk, Wv, bv)
    in_maps = _shard_maps(states, hosts)
    res = run_bass_kernel_spmd(nc, in_maps, list(range(NCORES)))
    out = np.empty((B, T, H), np.float32)
    for i in range(NCORES):
        b, hf = i // 2, i % 2
        out[b, hf * TC:(hf + 1) * TC] = res.results[i]["out"]
    return out



# revision 2
# speedup vs baseline: 1.0822x; 1.0822x over previous
"""Local causal (sliding-window) attention kernel for Trainium2, SPMD over 8 cores.

Problem: states [4, 4096, 1024] f32; q/k/v = states @ W*.T + b*; each query t
attends keys t-8..t (window=8), softmax over valid positions, out = attn @ v.

Sharding: data-parallel, 8 shards = 4 batches x 2 sequence halves (2048 queries
each). Each shard's states arrive pre-transposed and chunk-packed as
[128, 8, 2056] with an 8-col halo at the sequence start (zeros for the first
half; real previous-half tokens for the second half).

Score reformulation (saves one full GEMM): q.k = x_t^T A x_k + u[k] + const
with A = (Wq/sqrt(H))^T Wk precomputed on host. The device computes
Y = A @ X (one GEMM); scores come out TRANSPOSED (keys on partitions):
S^T_i = Y[:, frame_i]^T @ X[:, queries_i], which feeds softmax along the
partition dim with no transposes: exp bias = u[key] (per-partition), band
mask applied multiplicatively after exp, row-sum via a PE matmul against a
ones column, and P^T is directly the PV lhsT.

Tiling: 17 full tiles of 120 queries + 1 tail tile of 8. Each 120-query tile's
9-key windows span exactly 128 keys -> one sliding V frame per tile, so PV is
2 matmuls (plus the N=1 rowsum). V frames are recomputed on the 8-col overlap
(+6% V GEMM); the tail tile's 16-key V frame comes from the host (vtail).

Schedule: PE warm-up dummies during the initial weight DMA (HAM), Y GEMM
first (5 chunks, first small to shorten the DMA critical path), then a
per-frame software pipeline V_i | S^T_i | PV_{i-1} so the tail is one chain.
Inputs ride two HWDGE rings (x on sync, a/wv on scalar); outputs on gpsimd.
"""

import numpy as np
import ml_dtypes

import concourse.bacc as bacc
import concourse.mybir as mybir
import concourse.tile as tile
from concourse.bass_utils import run_bass_kernel_spmd

B, T, H = 4, 4096, 1024
NCORES = 8
TC = T // 2            # queries per core
HALO = 8               # window size
TH = TC + HALO         # x cols incl. halo
QT = 120               # queries per full tile (window spans exactly 128 keys)
NFT = 17               # full tiles; tail tile has TC - 17*120 = 8 queries
NTILE = NFT + 1
NQ_TAIL = TC - QT * NFT            # 8
NK_TAIL = NQ_TAIL + HALO           # 16
HC = H // 128          # 128-row chunks of H
NWARM = 20             # HAM warm-up dummy matmuls
# Y GEMM x-col chunks; first small so PE can start after ~2.5MB of DMA
YCHUNKS = [(8, 264), (264, 776), (776, 1288), (1288, 1800), (1800, 2056)]
XSEGS = [(0, 264), (264, 776), (776, 1288), (1288, 1800), (1800, 2056)]
F32 = mybir.dt.float32
BF16 = mybir.dt.bfloat16
BF = ml_dtypes.bfloat16
AF = mybir.ActivationFunctionType

_cache = {}


def _emit(nc, tc, aps, pools):
    (x_d, a_d, wv_d, bv_d, m0_d, mr_d, u2_d, yh_d, vt_d, out_d) = aps
    consts, xw, acts, attn, psY, psS, psO, psR = pools

    warm = consts.tile([128, 512], BF16, tag="warm", name="warm")
    ones_t = consts.tile([128, 1], BF16, tag="ones", name="ones_t")
    bv_t = consts.tile([128, H], BF16, tag="bv", name="bv_t")
    m0_t = consts.tile([128, QT], BF16, tag="m0", name="m0_t")
    mr_t = consts.tile([128, QT], BF16, tag="mr", name="mr_t")
    u2_t = consts.tile([128, NTILE], F32, tag="u2", name="u2_t")
    vtail_t = consts.tile([NK_TAIL, H], BF16, tag="vtail", name="vtail_t")

    x_all = xw.tile([128, HC, TH], BF16, tag="x", name="x_all")
    a_all = xw.tile([128, HC, H], BF16, tag="a", name="a_all")
    wv_all = xw.tile([128, HC, H], BF16, tag="wv", name="wv_all")
    y_all = acts.tile([128, HC, TH], BF16, tag="y", name="y_all")
    vt = [acts.tile([128, H], BF16, tag=f"v{i}", name=f"v{i}")
          for i in range(NFT)]

    # ---- DMA issue. sync ring: x segs then smalls; scalar ring: a, wv. ----
    for lo, hi in XSEGS:
        nc.sync.dma_start(x_all[:, :, lo:hi], x_d[:, :, lo:hi])
    nc.scalar.dma_start(a_all[:], a_d[:])
    nc.scalar.dma_start(wv_all[:], wv_d[:])
    nc.sync.dma_start(y_all[:, :, 0:HALO], yh_d[:])
    nc.sync.dma_start(bv_t[:], bv_d[:])
    nc.sync.dma_start(m0_t[:], m0_d[:])
    nc.sync.dma_start(mr_t[:], mr_d[:])
    nc.sync.dma_start(u2_t[:], u2_d[:])
    nc.sync.dma_start(vtail_t[:], vt_d[:])

    # ---- PE warm-up on a zeroed tile while weights stream in (HAM) ----
    nc.vector.memset(warm[:], 0.0)
    nc.vector.memset(ones_t[:], 1.0)
    for _ in range(NWARM):
        ps = psY.tile([128, 512], F32, tag="ps", name="ps_warm")
        nc.tensor.matmul(ps[:], warm[:, 0:128], warm[:], start=True, stop=True)

    # ---- Y = A @ X over all x cols (halo cols from host) ----
    for ci, (lo, hi) in enumerate(YCHUNKS):
        for hc in range(HC):
            ps = psY.tile([128, hi - lo], F32, tag="ps", name="ps_y")
            for c in range(HC):
                nc.tensor.matmul(ps[:], a_all[:, c, hc * 128:(hc + 1) * 128],
                                 x_all[:, c, lo:hi],
                                 start=(c == 0), stop=(c == HC - 1))
            if hc % 2 == 0:
                nc.scalar.copy(y_all[:, hc, lo:hi], ps[:])
            else:
                nc.vector.tensor_copy(y_all[:, hc, lo:hi], ps[:])

    # ---- V frames + attention, software-pipelined per frame ----
    pm_tiles = {}
    rq = [QT] * NFT + [NQ_TAIL]
    rk = [128] * NFT + [NK_TAIL]

    def emit_v(i):
        for hh in range(2):
            ps = psY.tile([128, 512], F32, tag="ps", name="ps_v")
            for c in range(HC):
                nc.tensor.matmul(ps[:], x_all[:, c, QT * i: QT * i + 128],
                                 wv_all[:, c, hh * 512:(hh + 1) * 512],
                                 start=(c == 0), stop=(c == HC - 1))
            nc.vector.tensor_add(vt[i][:, hh * 512:(hh + 1) * 512], ps[:],
                                 bv_t[:, hh * 512:(hh + 1) * 512])

    def emit_s(i):
        nq, nk, f0 = rq[i], rk[i], QT * i
        s_ps = psS.tile([128, QT], F32, tag="s", name="s_ps")
        for c in range(HC):
            nc.tensor.matmul(s_ps[:nk, :nq], y_all[:, c, f0:f0 + nk],
                             x_all[:, c, f0 + HALO:f0 + HALO + nq],
                             start=(c == 0), stop=(c == HC - 1))
        p = attn.tile([128, QT], BF16, tag="p", name="p")
        nc.scalar.activation(p[:nk, :nq], s_ps[:nk, :nq], AF.Exp,
                             bias=u2_t[0:nk, i:i + 1], scale=1.0)
        pm = attn.tile([128, QT], BF16, tag="pm", name="pm")
        mask = m0_t if i == 0 else mr_t
        nc.vector.tensor_mul(pm[:nk, :nq], p[:nk, :nq], mask[0:nk, 0:nq])
        pm_tiles[i] = pm

    def emit_pv(i):
        nq, nk = rq[i], rk[i]
        pm = pm_tiles.pop(i)
        vsrc = vt[i] if i < NFT else vtail_t
        rs_ps = psR.tile([QT, 1], F32, tag="rs", name="rs_ps")
        nc.tensor.matmul(rs_ps[:nq, :], pm[:nk, :nq], ones_t[0:nk, :],
                         start=True, stop=True)
        rinv = attn.tile([QT, 1], F32, tag="ri", name="rinv")
        nc.vector.reciprocal(rinv[:nq, :], rs_ps[:nq, :])
        out_sb = attn.tile([QT, H], F32, tag="osb", name="out_sb")
        for hh in range(2):
            o_ps = psO.tile([QT, 512], F32, tag="o", name="o_ps")
            nc.tensor.matmul(o_ps[:nq, :], pm[:nk, :nq],
                             vsrc[0:nk, hh * 512:(hh + 1) * 512],
                             start=True, stop=True)
            if hh == 0:
                nc.scalar.activation(out_sb[:nq, 0:512], o_ps[:nq, :],
                                     AF.Copy, bias=0.0, scale=rinv[:nq, :])
            else:
                nc.vector.tensor_scalar_mul(out_sb[:nq, 512:H], o_ps[:nq, :],
                                            rinv[:nq, :])
        nc.gpsimd.dma_start(out_d[QT * i: QT * i + nq, :], out_sb[:nq, :])

    for i in range(NTILE):
        if i < NFT:
            emit_v(i)
        emit_s(i)
        if i >= 1:
            emit_pv(i - 1)
    emit_pv(NTILE - 1)


def _build(loop_reps=None, trace_sim=False):
    key = ("nc", loop_reps, trace_sim)
    if key in _cache:
        return _cache[key]
    nc = bacc.Bacc("TRN2", target_bir_lowering=False, debug=False,
                   num_devices=NCORES)

    aps = (
        nc.dram_tensor("x", [128, HC, TH], BF16, kind="ExternalInput").ap(),
        nc.dram_tensor("a", [128, HC, H], BF16, kind="ExternalInput").ap(),
        nc.dram_tensor("wv", [128, HC, H], BF16, kind="ExternalInput").ap(),
        nc.dram_tensor("bv", [128, H], BF16, kind="ExternalInput").ap(),
        nc.dram_tensor("m0", [128, QT], BF16, kind="ExternalInput").ap(),
        nc.dram_tensor("mr", [128, QT], BF16, kind="ExternalInput").ap(),
        nc.dram_tensor("u2", [128, NTILE], F32, kind="ExternalInput").ap(),
        nc.dram_tensor("yhalo", [128, HC, HALO], BF16,
                       kind="ExternalInput").ap(),
        nc.dram_tensor("vtail", [NK_TAIL, H], BF16, kind="ExternalInput").ap(),
        nc.dram_tensor("out", [TC, H], F32, kind="ExternalOutput").ap(),
    )

    with tile.TileContext(nc, trace_sim=trace_sim) as tc:
        with (
            tc.tile_pool(name="consts", bufs=1) as consts,
            tc.tile_pool(name="xw", bufs=1) as xw,
            tc.tile_pool(name="acts", bufs=1) as acts,
            tc.tile_pool(name="attn", bufs=3) as attn,
            tc.tile_pool(name="psY", bufs=3, space="PSUM") as psY,
            tc.tile_pool(name="psS", bufs=2, space="PSUM") as psS,
            tc.tile_pool(name="psO", bufs=2, space="PSUM") as psO,
            tc.tile_pool(name="psR", bufs=1, space="PSUM") as psR,
        ):
            pools = (consts, xw, acts, attn, psY, psS, psO, psR)
            if loop_reps:
                with tc.For_i(0, loop_reps, 1):
                    _emit(nc, tc, aps, pools)
            else:
                _emit(nc, tc, aps, pools)

    nc.compile()
    _cache[key] = nc
    return nc


def _pack(m):
    """[128*HC, W] row-chunked -> [128, HC, W] (partition-major packing)."""
    w = m.shape[1]
    return np.ascontiguousarray(
        m.reshape(HC, 128, w).transpose(1, 0, 2))


def _host_inputs(states, Wq, bq, Wk, bk, Wv, bv):
    """Shared (per-run) host-side tensor prep."""
    scale = 1.0 / np.sqrt(H)
    Wq = np.asarray(Wq, np.float32)
    Wk = np.asarray(Wk, np.float32)
    Wv = np.asarray(Wv, np.float32)
    bq = np.asarray(bq, np.float32)
    bv = np.asarray(bv, np.float32)
    Wqs = Wq * scale
    # A = Wqs.T @ Wk ; device lhsT layout needs A.T = Wk.T @ Wqs
    at_h = np.ascontiguousarray(Wk.T @ Wqs).astype(BF)
    # per-key rank-1 vector; per-query term and constants cancel in softmax
    wt_h = Wk.T @ (bq * scale)
    wv_h = np.ascontiguousarray(Wv.T).astype(BF)
    a_p = _pack(at_h.astype(BF))
    wv_p = _pack(wv_h.astype(BF))
    bv_h = np.ascontiguousarray(np.broadcast_to(bv, (128, H))).astype(BF)
    k = np.arange(128)[:, None]
    t = np.arange(QT)[None, :]
    band = (k >= t) & (k <= t + HALO)
    mr_h = band.astype(BF)
    m0_h = (band & (k >= HALO)).astype(BF)
    return at_h, wt_h, wv_p, bv_h, m0_h, mr_h, a_p, bv


def _shard_maps(states, hosts):
    at_h, wt_h, wv_p, bv_h, m0_h, mr_h, a_p, bv = hosts
    a_f = at_h.astype(np.float32)      # A.T in bf16 precision
    wv_f = wv_p.transpose(1, 0, 2).reshape(H, H).astype(np.float32)  # Wv.T
    in_maps = []
    for i in range(NCORES):
        b, hf = i // 2, i % 2
        xs = np.zeros((TH, H), np.float32)
        if hf == 0:
            xs[HALO:] = states[b, 0:TC]
        else:
            xs[:] = states[b, TC - HALO: 2 * TC]
        x_h = np.ascontiguousarray(xs.T).astype(BF)   # [H, TH]
        x_f = x_h.astype(np.float32)
        u_full = wt_h @ x_f                            # [TH]
        u2 = np.zeros((128, NTILE), np.float32)
        for j in range(NFT):
            u2[:, j] = u_full[QT * j: QT * j + 128]
        u2[:NK_TAIL, NFT] = u_full[QT * NFT: QT * NFT + NK_TAIL]
        yh = (a_f.T @ x_f[:, :HALO])                   # [H, 8] = A @ x_halo
        vtail_h = (x_f[:, QT * NFT:].T @ wv_f + bv).astype(BF)  # [16, H]
        in_maps.append({
            "x": _pack(x_h), "a": a_p, "wv": wv_p, "bv": bv_h,
            "m0": (m0_h if hf == 0 else mr_h), "mr": mr_h,
            "u2": u2, "yhalo": _pack(yh.astype(BF)), "vtail": vtail_h,
        })
    return in_maps


def kernel(states, Wq, bq, Wk, bk, Wv, bv, window):
    assert int(window) == HALO
    states = np.asarray(states, np.float32)
    nc = _build()
    hosts = _host_inputs(states, Wq, bq, Wk, bk, Wv, bv)
    in_maps = _shard_maps(states, hosts)
    res = run_bass_kernel_spmd(nc, in_maps, list(range(NCORES)))
    out = np.empty((B, T, H), np.float32)
    for i in range(NCORES):
        b, hf = i // 2, i % 2
        out[b, hf * TC:(hf + 1) * TC] = res.results[i]["out"]
    return out


# revision 7
# speedup vs baseline: 1.1042x; 1.0203x over previous
"""Local causal (sliding-window) attention kernel for Trainium2, SPMD over 8 cores.

Problem: states [4, 4096, 1024] f32; q/k/v = states @ W*.T + b*; each query t
attends keys t-8..t (window=8), softmax over valid positions, out = attn @ v.

Sharding: data-parallel, 8 shards = 4 batches x 2 sequence halves (2048 queries
each). Each shard's states arrive pre-transposed and chunk-packed as
[128, 8, 2056] with an 8-col halo at the sequence start (zeros for the first
half; real previous-half tokens for the second half).

Score reformulation (saves one full GEMM): q.k = x_t^T A x_k + u[k] + const
with A = (Wq/sqrt(H))^T Wk precomputed on host. The device computes
Y = A @ X (one GEMM); scores come out TRANSPOSED (keys on partitions):
S^T_i = Y[:, frame_i]^T @ X[:, queries_i], which feeds softmax along the
partition dim with no transposes: exp bias = u[key] (per-partition), band
mask applied multiplicatively after exp, row-sum via a PE matmul against a
ones column, and P^T is directly the PV lhsT.

Tiling: 17 full tiles of 120 queries + 1 tail tile of 8. Each 120-query tile's
9-key windows span exactly 128 keys -> one sliding V frame per tile, so PV is
2 matmuls (plus the N=1 rowsum). V frames are recomputed on the 8-col overlap
(+6% V GEMM); the tail tile's 16-key V frame comes from the host (vtail).

Schedule: PE warm-up dummies during the initial weight DMA (HAM), Y GEMM
first (5 chunks, first small to shorten the DMA critical path), then a
per-frame software pipeline V_i | S^T_i | PV_{i-1} so the tail is one chain.
Inputs ride two HWDGE rings (x on sync, a/wv on scalar); outputs on gpsimd.
"""

import numpy as np
import ml_dtypes

import concourse.bacc as bacc
import concourse.mybir as mybir
import concourse.tile as tile
from concourse.bass_utils import run_bass_kernel_spmd

B, T, H = 4, 4096, 1024
NCORES = 8
TC = T // 2            # queries per core
HALO = 8               # window size
TH = TC + HALO         # x cols incl. halo
QT = 120               # queries per full tile (window spans exactly 128 keys)
NFT = 17               # full tiles; tail tile has TC - 17*120 = 8 queries
NTILE = NFT + 1
NQ_TAIL = TC - QT * NFT            # 8
NK_TAIL = NQ_TAIL + HALO           # 16
HC = H // 128          # 128-row chunks of H
NWARM = 20             # HAM warm-up dummy matmuls
# Y GEMM x-col chunks; first small so PE can start after ~2.5MB of DMA
YCHUNKS = [(8, 264), (264, 776), (776, 1288), (1288, 1800), (1800, 2056)]
XSEGS = [(0, 264), (264, 776), (776, 1288), (1288, 1800), (1800, 2056)]
F32 = mybir.dt.float32
BF16 = mybir.dt.bfloat16
BF = ml_dtypes.bfloat16
AF = mybir.ActivationFunctionType

_cache = {}


def _emit(nc, tc, aps, pools):
    (xs_d, a_d, wv_d, bv_d, m0_d, mr_d, u2_d, yh_d, vt_d, out_d) = aps
    consts, xw, acts, attn, psY, psS, psO, psR = pools

    warm = consts.tile([128, 512], BF16, tag="warm", name="warm")
    ones_t = consts.tile([128, 1], BF16, tag="ones", name="ones_t")
    bv_t = consts.tile([128, H], BF16, tag="bv", name="bv_t")
    m0_t = consts.tile([128, QT], BF16, tag="m0", name="m0_t")
    mr_t = consts.tile([128, QT], BF16, tag="mr", name="mr_t")
    u2_t = consts.tile([128, NTILE], F32, tag="u2", name="u2_t")
    vtail_t = consts.tile([NK_TAIL, H], BF16, tag="vtail", name="vtail_t")

    x_all = xw.tile([128, HC, TH], BF16, tag="x", name="x_all")
    a_all = xw.tile([128, HC, H], BF16, tag="a", name="a_all")
    wv_all = xw.tile([128, HC, H], BF16, tag="wv", name="wv_all")
    y_all = acts.tile([128, HC, TH], BF16, tag="y", name="y_all")
    vt = [acts.tile([128, H], BF16, tag=f"v{i}", name=f"v{i}")
          for i in range(NFT)]

    # ---- DMA issue. Critical path (xseg0, a) rides the scalar HWDGE ring
    # (starts earlier, FIFO within ring); the rest on the sync ring. ----
    nc.scalar.dma_start(x_all[:, :, XSEGS[0][0]:XSEGS[0][1]], xs_d[0][:])
    nc.scalar.dma_start(a_all[:], a_d[:])
    nc.scalar.dma_start(wv_all[:], wv_d[:])
    for s in range(1, len(XSEGS)):
        lo, hi = XSEGS[s]
        nc.sync.dma_start(x_all[:, :, lo:hi], xs_d[s][:])
    nc.sync.dma_start(y_all[:, :, 0:HALO], yh_d[:])
    nc.sync.dma_start(bv_t[:], bv_d[:])
    nc.sync.dma_start(m0_t[:], m0_d[:])
    nc.sync.dma_start(mr_t[:], mr_d[:])
    nc.sync.dma_start(u2_t[:], u2_d[:])
    nc.sync.dma_start(vtail_t[:], vt_d[:])

    # ---- PE warm-up on a zeroed tile while weights stream in (HAM) ----
    nc.vector.memset(warm[:], 0.0)
    nc.vector.memset(ones_t[:], 1.0)
    for _ in range(NWARM):
        ps = psY.tile([128, 512], F32, tag="ps", name="ps_warm")
        nc.tensor.matmul(ps[:], warm[:, 0:128], warm[:], start=True, stop=True)

    # ---- Y = A @ X over all x cols (halo cols from host) ----
    for ci, (lo, hi) in enumerate(YCHUNKS):
        for hc in range(HC):
            ps = psY.tile([128, hi - lo], F32, tag="ps", name="ps_y")
            for c in range(HC):
                nc.tensor.matmul(ps[:], a_all[:, c, hc * 128:(hc + 1) * 128],
                                 x_all[:, c, lo:hi],
                                 start=(c == 0), stop=(c == HC - 1))
            if hc % 2 == 0:
                nc.scalar.copy(y_all[:, hc, lo:hi], ps[:])
            else:
                nc.vector.tensor_copy(y_all[:, hc, lo:hi], ps[:])

    # ---- V frames + attention, software-pipelined per frame ----
    pm_tiles = {}
    rq = [QT] * NFT + [NQ_TAIL]
    rk = [128] * NFT + [NK_TAIL]

    def emit_v(i):
        for hh in range(2):
            ps = psY.tile([128, 512], F32, tag="ps", name="ps_v")
            for c in range(HC):
                nc.tensor.matmul(ps[:], x_all[:, c, QT * i: QT * i + 128],
                                 wv_all[:, c, hh * 512:(hh + 1) * 512],
                                 start=(c == 0), stop=(c == HC - 1))
            nc.vector.tensor_add(vt[i][:, hh * 512:(hh + 1) * 512], ps[:],
                                 bv_t[:, hh * 512:(hh + 1) * 512])

    def emit_s(i):
        nq, nk, f0 = rq[i], rk[i], QT * i
        s_ps = psS.tile([128, QT], F32, tag="s", name="s_ps")
        for c in range(HC):
            nc.tensor.matmul(s_ps[:nk, :nq], y_all[:, c, f0:f0 + nk],
                             x_all[:, c, f0 + HALO:f0 + HALO + nq],
                             start=(c == 0), stop=(c == HC - 1))
        p = attn.tile([128, QT], BF16, tag="p", name="p")
        nc.scalar.activation(p[:nk, :nq], s_ps[:nk, :nq], AF.Exp,
                             bias=u2_t[0:nk, i:i + 1], scale=1.0)
        pm = attn.tile([128, QT], BF16, tag="pm", name="pm")
        mask = m0_t if i == 0 else mr_t
        nc.vector.tensor_mul(pm[:nk, :nq], p[:nk, :nq], mask[0:nk, 0:nq])
        pm_tiles[i] = pm

    def emit_pv(i):
        nq, nk = rq[i], rk[i]
        pm = pm_tiles.pop(i)
        vsrc = vt[i] if i < NFT else vtail_t
        rs_ps = psR.tile([QT, 1], F32, tag="rs", name="rs_ps")
        nc.tensor.matmul(rs_ps[:nq, :], pm[:nk, :nq], ones_t[0:nk, :],
                         start=True, stop=True)
        rinv = attn.tile([QT, 1], F32, tag="ri", name="rinv")
        nc.vector.reciprocal(rinv[:nq, :], rs_ps[:nq, :])
        out_sb = attn.tile([QT, H], F32, tag="osb", name="out_sb")
        for hh in range(2):
            o_ps = psO.tile([QT, 512], F32, tag="o", name="o_ps")
            nc.tensor.matmul(o_ps[:nq, :], pm[:nk, :nq],
                             vsrc[0:nk, hh * 512:(hh + 1) * 512],
                             start=True, stop=True)
            if hh == 0:
                nc.scalar.activation(out_sb[:nq, 0:512], o_ps[:nq, :],
                                     AF.Copy, bias=0.0, scale=rinv[:nq, :])
            else:
                nc.vector.tensor_scalar_mul(out_sb[:nq, 512:H], o_ps[:nq, :],
                                            rinv[:nq, :])
        nc.sync.dma_start(out_d[QT * i: QT * i + nq, :], out_sb[:nq, :])

    for i in range(NTILE):
        if i < NFT:
            emit_v(i)
        emit_s(i)
        if i >= 1:
            emit_pv(i - 1)
    emit_pv(NTILE - 1)


def _build(loop_reps=None, trace_sim=False):
    key = ("nc", loop_reps, trace_sim)
    if key in _cache:
        return _cache[key]
    nc = bacc.Bacc("TRN2", target_bir_lowering=False, debug=False,
                   num_devices=NCORES)

    aps = (
        [nc.dram_tensor(f"x{s}", [128, HC, hi - lo], BF16,
                        kind="ExternalInput").ap()
         for s, (lo, hi) in enumerate(XSEGS)],
        nc.dram_tensor("a", [128, HC, H], BF16, kind="ExternalInput").ap(),
        nc.dram_tensor("wv", [128, HC, H], BF16, kind="ExternalInput").ap(),
        nc.dram_tensor("bv", [128, H], BF16, kind="ExternalInput").ap(),
        nc.dram_tensor("m0", [128, QT], BF16, kind="ExternalInput").ap(),
        nc.dram_tensor("mr", [128, QT], BF16, kind="ExternalInput").ap(),
        nc.dram_tensor("u2", [128, NTILE], F32, kind="ExternalInput").ap(),
        nc.dram_tensor("yhalo", [128, HC, HALO], BF16,
                       kind="ExternalInput").ap(),
        nc.dram_tensor("vtail", [NK_TAIL, H], BF16, kind="ExternalInput").ap(),
        nc.dram_tensor("out", [TC, H], F32, kind="ExternalOutput").ap(),
    )

    with tile.TileContext(nc, trace_sim=trace_sim) as tc:
        with (
            tc.tile_pool(name="consts", bufs=1) as consts,
            tc.tile_pool(name="xw", bufs=1) as xw,
            tc.tile_pool(name="acts", bufs=1) as acts,
            tc.tile_pool(name="attn", bufs=3) as attn,
            tc.tile_pool(name="psY", bufs=3, space="PSUM") as psY,
            tc.tile_pool(name="psS", bufs=2, space="PSUM") as psS,
            tc.tile_pool(name="psO", bufs=2, space="PSUM") as psO,
            tc.tile_pool(name="psR", bufs=1, space="PSUM") as psR,
        ):
            pools = (consts, xw, acts, attn, psY, psS, psO, psR)
            if loop_reps:
                with tc.For_i(0, loop_reps, 1):
                    _emit(nc, tc, aps, pools)
            else:
                _emit(nc, tc, aps, pools)

    nc.compile()
    _cache[key] = nc
    return nc


def _pack(m):
    """[128*HC, W] row-chunked -> [128, HC, W] (partition-major packing)."""
    w = m.shape[1]
    return np.ascontiguousarray(
        m.reshape(HC, 128, w).transpose(1, 0, 2))


def _host_inputs(states, Wq, bq, Wk, bk, Wv, bv):
    """Shared (per-run) host-side tensor prep."""
    scale = 1.0 / np.sqrt(H)
    Wq = np.asarray(Wq, np.float32)
    Wk = np.asarray(Wk, np.float32)
    Wv = np.asarray(Wv, np.float32)
    bq = np.asarray(bq, np.float32)
    bv = np.asarray(bv, np.float32)
    Wqs = Wq * scale
    # A = Wqs.T @ Wk ; device lhsT layout needs A.T = Wk.T @ Wqs
    at_h = np.ascontiguousarray(Wk.T @ Wqs).astype(BF)
    # per-key rank-1 vector; per-query term and constants cancel in softmax
    wt_h = Wk.T @ (bq * scale)
    wv_h = np.ascontiguousarray(Wv.T).astype(BF)
    a_p = _pack(at_h.astype(BF))
    wv_p = _pack(wv_h.astype(BF))
    bv_h = np.ascontiguousarray(np.broadcast_to(bv, (128, H))).astype(BF)
    k = np.arange(128)[:, None]
    t = np.arange(QT)[None, :]
    band = (k >= t) & (k <= t + HALO)
    mr_h = band.astype(BF)
    m0_h = (band & (k >= HALO)).astype(BF)
    return at_h, wt_h, wv_p, bv_h, m0_h, mr_h, a_p, bv


def _shard_maps(states, hosts):
    at_h, wt_h, wv_p, bv_h, m0_h, mr_h, a_p, bv = hosts
    a_f = at_h.astype(np.float32)      # A.T in bf16 precision
    wv_f = wv_p.transpose(1, 0, 2).reshape(H, H).astype(np.float32)  # Wv.T
    in_maps = []
    for i in range(NCORES):
        b, hf = i // 2, i % 2
        xs = np.zeros((TH, H), np.float32)
        if hf == 0:
            xs[HALO:] = states[b, 0:TC]
        else:
            xs[:] = states[b, TC - HALO: 2 * TC]
        x_h = np.ascontiguousarray(xs.T).astype(BF)   # [H, TH]
        x_f = x_h.astype(np.float32)
        u_full = wt_h @ x_f                            # [TH]
        u2 = np.zeros((128, NTILE), np.float32)
        for j in range(NFT):
            u2[:, j] = u_full[QT * j: QT * j + 128]
        u2[:NK_TAIL, NFT] = u_full[QT * NFT: QT * NFT + NK_TAIL]
        yh = (a_f.T @ x_f[:, :HALO])                   # [H, 8] = A @ x_halo
        vtail_h = (x_f[:, QT * NFT:].T @ wv_f + bv).astype(BF)  # [16, H]
        im = {
            "a": a_p, "wv": wv_p, "bv": bv_h,
            "m0": (m0_h if hf == 0 else mr_h), "mr": mr_h,
            "u2": u2, "yhalo": _pack(yh.astype(BF)), "vtail": vtail_h,
        }
        for s, (lo, hi) in enumerate(XSEGS):
            im[f"x{s}"] = _pack(x_h[:, lo:hi])
        in_maps.append(im)
    return in_maps


def kernel(states, Wq, bq, Wk, bk, Wv, bv, window):
    assert int(window) == HALO
    states = np.asarray(states, np.float32)
    nc = _build()
    hosts = _host_inputs(states, Wq, bq, Wk, bk, Wv, bv)
    in_maps = _shard_maps(states, hosts)
    res = run_bass_kernel_spmd(nc, in_maps, list(range(NCORES)))
    out = np.empty((B, T, H), np.float32)
    for i in range(NCORES):
        b, hf = i // 2, i % 2
        out[b, hf * TC:(hf + 1) * TC] = res.results[i]["out"]
    return out
